# revision 13
# baseline (speedup 1.0000x reference)
"""Trainium2 Bass kernel for GCMC-style GNN message passing (nn_Net_6425271075083).

Strategy (8 NeuronCores, users sharded 1250/core):
  - Host converts the edge lists into dense per-rating adjacency count
    matrices (counts <= 3, exact in bf16) and the implicit-feedback
    index matrix into a per-user histogram; degrees -> cu/ci norm vectors.
  - Device does all the dense math:
      item side:  M_c^T = sum_r (D_cu (ufeat_c @ W_r))^T @ A_r[users_c]   [256,1024]
                  -> AllReduce over 8 cores -> item_agg^T
      user side:  user_agg_c = sum_r A_r[users_c]^T-layout @ (D_ci (ifeat @ W_r))
      heads:      p^T = fc_w^T @ leaky(user_agg * cu)^T (PE transposes)
                  y^T = Y0^T @ (Hist_c / sqrt_count)^T   (fused in same PSUM)
                  q^T = fc_w^T @ leaky(item_agg^T)  (ci deferred to final evict)
      final:      out_c = D_ci (q'^T.T @ s'^T) + bi, with the extra
                  q' row = 1/ci and s' row = bu + gm carrying the bias terms.
  - Big matmuls run as split-bf16 (hi+lo) against the exact-bf16 adjacency:
    validated rel-L2 vs fp32 reference ~6.5e-8.
"""
import os
import numpy as np
import ml_dtypes

import concourse.bass as bass
import concourse.bacc as bacc
import concourse.mybir as mybir
import concourse.tile as tile
from concourse import bass_utils
from concourse.masks import make_identity

BF = ml_dtypes.bfloat16
F32 = mybir.dt.float32
BF16 = mybir.dt.bfloat16

N_CORES = 8
U, I, R, D, O, H = 10000, 1000, 5, 256, 64, 1001
UC = U // N_CORES          # 1250
UCP = 1280                 # users per core, padded
IP = 1024                  # items padded
HP = 1024                  # hist bins padded
KU = UCP // 128            # 10 user k/m tiles
KI = IP // 128             # 8 item k/m tiles
KH = HP // 128             # 8 hist k tiles
UCHUNKS = [(0, 512), (512, 512), (1024, 256)]   # user free-dim chunks (padded)
ICHUNKS = [(0, 512), (512, 512)]                # item free-dim chunks

_ALU = mybir.AluOpType


def _split_bf16(x):
    hi = x.astype(BF)
    lo = (x - hi.astype(np.float32)).astype(BF)
    return hi, lo


def host_preprocess(src_idx, dst_idx, implicit_matrix, sqrt_count, global_mean,
                    ufeat, ifeat, W, fc_w, fc_b, bu, bi, Y):
    """Build per-core input maps (layout/sharding only plus degree/adjacency
    densification; all NN math happens on device)."""
    src = np.asarray(src_idx).astype(np.int64)
    dst = np.asarray(dst_idx).astype(np.int64)
    im = np.asarray(implicit_matrix).astype(np.int64)
    sqrt_count = np.asarray(sqrt_count, np.float32)
    gm = np.asarray(global_mean, np.float32).reshape(1)
    ufeat = np.asarray(ufeat, np.float32)
    ifeat = np.asarray(ifeat, np.float32)
    W = np.asarray(W, np.float32)
    fc_w = np.asarray(fc_w, np.float32)
    fc_b = np.asarray(fc_b, np.float32)
    bu = np.asarray(bu, np.float32)
    bi = np.asarray(bi, np.float32)
    Y = np.asarray(Y, np.float32)

    deg_u = np.bincount(src.reshape(-1), minlength=U).astype(np.float32)
    deg_i = np.bincount(dst.reshape(-1), minlength=I).astype(np.float32)
    cu = 1.0 / np.sqrt(np.maximum(deg_u, 1.0))
    ci = 1.0 / np.sqrt(np.maximum(deg_i, 1.0))
    def pack_cols(vec, ntiles):
        out = np.zeros((128, ntiles), np.float32)
        padded = np.zeros(128 * ntiles, np.float32)
        padded[:len(vec)] = vec
        out[:] = padded.reshape(ntiles, 128).T
        return out

    ci2 = pack_cols(ci, KI)
    bi2 = pack_cols(bi[:, 0], KI)
    cirecip_row = np.zeros((1, IP), np.float32)
    cirecip_row[0, :I] = 1.0 / ci

    # dense adjacency counts per rating [U, I]
    G = np.zeros((R, U, I), np.float32)
    for r in range(R):
        G[r] = np.bincount(src[r] * I + dst[r], minlength=U * I).reshape(U, I)

    # implicit histogram [U, H] with 1/sqrt_count folded
    hist = np.bincount((np.arange(U)[:, None] * H + im).reshape(-1),
                       minlength=U * H).reshape(U, H).astype(np.float32)
    histp = hist / sqrt_count

    Y0 = Y.copy()
    Y0[0] = 0.0
    y0_t = np.zeros((HP, O), np.float32)
    y0_t[:H] = Y0
    y0_t = y0_t.reshape(KH, 128, O)

    wh, wl = _split_bf16(W)                       # [5,256,256]
    ifT = np.zeros((D, IP), np.float32)
    ifT[:, :I] = ifeat.T
    ifh, ifl = _split_bf16(ifT)

    in_maps = []
    for c in range(N_CORES):
        us = slice(c * UC, (c + 1) * UC)
        # ga: [R, KU, 128, IP]  (lhs/rhs layout [users, items])
        ga = np.zeros((R, UCP, IP), BF)
        for r in range(R):
            ga[r, :UC, :I] = G[r][us].astype(BF)
        ga = ga.reshape(R, KU, 128, IP)
        # gb: [KU(m), R, 128(p=item-in-tile), KI*128(u)] from G^T
        gb = np.zeros((KU, R, 128, KI * 128), BF)
        for r in range(R):
            gt = np.zeros((IP, UCP), np.float32)
            gt[:I, :UC] = G[r][us].T
            # block for user-tile m: [IP, 128] -> [p, k*128+u]
            blocks = gt.reshape(KI, 128, KU, 128).transpose(2, 1, 0, 3)
            # blocks[m, p, k, u]
            gb[:, r] = blocks.reshape(KU, 128, KI * 128).astype(BF)

        ufT = np.zeros((D, UCP), np.float32)
        ufT[:, :UC] = ufeat[us].T
        ufh, ufl = _split_bf16(ufT)

        cu2 = pack_cols(cu[us], KU)
        bu_row = np.zeros((1, UCP), np.float32)
        bu_row[0, :UC] = bu[us, 0]

        hist_t = np.zeros((HP, UCP), np.float32)
        hist_t[:H, :UC] = histp[us].T
        hist_t = hist_t.reshape(KH, 128, UCP)

        in_maps.append({
            "ga": ga, "gb": gb,
            "uft_h": ufh, "uft_l": ufl,
            "ift_h": ifh, "ift_l": ifl,
            "w_h": wh, "w_l": wl,
            "fcw": fc_w.reshape(2, 128, O).copy(),
            "fcb": fc_b.reshape(O, 1).copy(),
            "y0t": y0_t, "histt": hist_t,
            "cu2": cu2, "ci2": ci2, "bi2": bi2,
            "cirecip": cirecip_row, "bu_row": bu_row,
            "gmv": gm.reshape(1, 1).copy(),
        })
    return in_maps


def declare_io(nc):
    t = {}
    def inp(name, shape, dt):
        t[name] = nc.dram_tensor(name, list(shape), dt, kind="ExternalInput").ap()
    inp("ga", (R, KU, 128, IP), BF16)
    inp("gb", (KU, R, 128, KI * 128), BF16)
    inp("uft_h", (D, UCP), BF16); inp("uft_l", (D, UCP), BF16)
    inp("ift_h", (D, IP), BF16); inp("ift_l", (D, IP), BF16)
    inp("w_h", (R, D, D), BF16); inp("w_l", (R, D, D), BF16)
    inp("fcw", (2, 128, O), F32)
    inp("fcb", (O, 1), F32)
    inp("y0t", (KH, 128, O), F32)
    inp("histt", (KH, 128, UCP), F32)
    inp("cu2", (128, KU), F32); inp("ci2", (128, KI), F32)
    inp("bi2", (128, KI), F32)
    inp("cirecip", (1, IP), F32); inp("bu_row", (1, UCP), F32)
    inp("gmv", (1, 1), F32)
    t["out"] = nc.dram_tensor("out", [I, UC], F32, kind="ExternalOutput").ap()
    return t


def emit_body(nc, tc, t, it):
    """Emit one full compute pass. `it` suffixes tile names for repeats."""
    from contextlib import ExitStack
    ctx = ExitStack()
    P = 128

    const = ctx.enter_context(tc.tile_pool(name=f"const{it}", bufs=1))

    def load_const(name, shape, dt, src_ap):
        tl = const.tile(shape, dt, name=f"{name}{it}")
        nc.sync.dma_start(tl[:], src_ap)
        return tl

    ident = const.tile([P, P], F32, name=f"ident{it}")
    make_identity(nc, ident[:])

    cu2 = load_const("cu2", [P, KU], F32, t["cu2"][:])
    ci2 = load_const("ci2", [P, KI], F32, t["ci2"][:])
    bi2 = load_const("bi2", [P, KI], F32, t["bi2"][:])
    cirecip = load_const("cirecip", [1, IP], F32, t["cirecip"][:])
    bu_in = load_const("bu_in", [1, UCP], F32, t["bu_row"][:])
    gmv = load_const("gmv", [1, 1], F32, t["gmv"][:])
    fcb = load_const("fcb", [O, 1], F32, t["fcb"][:])
    fcw = [load_const(f"fcw{k}", [P, O], F32, t["fcw"][k]) for k in range(2)]
    y0 = [load_const(f"y0_{k}", [P, O], F32, t["y0t"][k]) for k in range(KH)]
    hist = [load_const(f"hist{k}", [P, UCP], F32, t["histt"][k]) for k in range(KH)]
    uft = {s: [load_const(f"uft{s}{k}", [P, UCP], BF16,
                          t[f"uft_{s}"][k * P:(k + 1) * P, :]) for k in range(2)]
           for s in ("h", "l")}
    ift = {s: [load_const(f"ift{s}{k}", [P, IP], BF16,
                          t[f"ift_{s}"][k * P:(k + 1) * P, :]) for k in range(2)]
           for s in ("h", "l")}
    w = {s: [[load_const(f"w{s}{r}_{k}", [P, D], BF16,
                         t[f"w_{s}"][r, k * P:(k + 1) * P, :]) for k in range(2)]
             for r in range(R)]
         for s in ("h", "l")}

    burow = const.tile([1, UCP], F32, name=f"burow{it}")
    nc.vector.tensor_scalar_add(burow[:], bu_in[:], gmv[:, 0:1])

    # ---------------- item phase ----------------
    ga_pool = ctx.enter_context(tc.tile_pool(name=f"ga{it}", bufs=3))
    xw_pool = ctx.enter_context(tc.tile_pool(name=f"xw{it}", bufs=3))
    from contextlib import ExitStack as _ES
    item_ctx = _ES()
    psx_pool = item_ctx.enter_context(tc.tile_pool(name=f"psx{it}", bufs=2, space="PSUM"))
    psb_pool = item_ctx.enter_context(tc.tile_pool(name=f"psb{it}", bufs=1, space="PSUM"))

    psB = [[psb_pool.tile([P, 512], F32, name=f"psB{h}{cix}{it}")
            for cix in range(2)] for h in range(2)]
    n_rk = R * KU
    rk = 0
    for r in range(R):
        for k in range(KU):
            psx = psx_pool.tile([P, D], F32, name=f"psx{it}")
            mms = [(uft["h"][kk], w["h"][r][kk]) for kk in range(2)] + \
                  [(uft["l"][kk], w["h"][r][kk]) for kk in range(2)] + \
                  [(uft["h"][kk], w["l"][r][kk]) for kk in range(2)]
            for i, (lh, rh) in enumerate(mms):
                nc.tensor.matmul(psx[:], lh[:, k * P:(k + 1) * P], rh[:],
                                 start=(i == 0), stop=(i == len(mms) - 1))
            z32 = xw_pool.tile([P, D], F32, name=f"z32{it}", tag="z32")
            nc.vector.tensor_scalar_mul(z32[:], psx[:], cu2[:, k:k + 1])
            xh = xw_pool.tile([P, D], BF16, name=f"xh{it}", tag="xh")
            nc.vector.tensor_copy(xh[:], z32[:])
            xl = xw_pool.tile([P, D], BF16, name=f"xl{it}", tag="xl")
            nc.vector.scalar_tensor_tensor(xl[:], xh[:], -1.0, z32[:],
                                           _ALU.mult, _ALU.add)
            ga_t = ga_pool.tile([P, IP], BF16, name=f"ga_t{it}")
            nc.sync.dma_start(ga_t[:], t["ga"][r, k])
            for h in range(2):
                for cix, (c0, cw) in enumerate(ICHUNKS):
                    for x in (xh, xl):
                        nc.tensor.matmul(
                            psB[h][cix][:], x[:, h * P:(h + 1) * P],
                            ga_t[:, c0:c0 + cw],
                            start=(rk == 0 and x is xh),
                            stop=(rk == n_rk - 1 and x is xl))
            rk += 1

    mcT = [const.tile([P, IP], F32, name=f"mcT{h}{it}") for h in range(2)]
    for h in range(2):
        for cix, (c0, cw) in enumerate(ICHUNKS):
            nc.vector.tensor_copy(mcT[h][:, c0:c0 + cw], psB[h][cix][:])

    dram = ctx.enter_context(tc.tile_pool(name=f"dram{it}", bufs=1, space="DRAM"))
    itemp = dram.tile([D, IP], F32, name=f"itemp{it}")
    itemagg = dram.tile([D, IP], F32, name=f"itemagg{it}", addr_space="Shared")
    for h in range(2):
        nc.sync.dma_start(itemp[h * P:(h + 1) * P, :], mcT[h][:])
    nc.gpsimd.collective_compute(
        "AllReduce", _ALU.add,
        replica_groups=[list(range(N_CORES))],
        ins=[itemp.opt()], outs=[itemagg.opt()],
    )
    item_ctx.close()

    # ---------------- user phase: hi ----------------
    user_ctx = _ES()
    psh_pool = user_ctx.enter_context(tc.tile_pool(name=f"psh{it}", bufs=2, space="PSUM"))
    hi_pool = ctx.enter_context(tc.tile_pool(name=f"hi{it}", bufs=2 * R * KI))
    z_pool = ctx.enter_context(tc.tile_pool(name=f"zu{it}", bufs=3))
    hi = {"h": {}, "l": {}}
    for r in range(R):
        for k in range(KI):
            psh = psh_pool.tile([P, D], F32, name=f"psh{it}")
            mms = [(ift["h"][kk], w["h"][r][kk]) for kk in range(2)] + \
                  [(ift["l"][kk], w["h"][r][kk]) for kk in range(2)] + \
                  [(ift["h"][kk], w["l"][r][kk]) for kk in range(2)]
            for i, (lh, rh) in enumerate(mms):
                nc.tensor.matmul(psh[:], lh[:, k * P:(k + 1) * P], rh[:],
                                 start=(i == 0), stop=(i == len(mms) - 1))
            z32 = z_pool.tile([P, D], F32, name=f"zh{it}", tag="zh")
            nc.vector.tensor_scalar_mul(z32[:], psh[:], ci2[:, k:k + 1])
            hh = hi_pool.tile([P, D], BF16, name=f"hih{r}_{k}{it}", tag="hi")
            nc.vector.tensor_copy(hh[:], z32[:])
            hl = hi_pool.tile([P, D], BF16, name=f"hil{r}_{k}{it}", tag="hi")
            nc.vector.scalar_tensor_tensor(hl[:], hh[:], -1.0, z32[:],
                                           _ALU.mult, _ALU.add)
            hi["h"][(r, k)] = hh
            hi["l"][(r, k)] = hl

    # ---------------- user phase: user_agg + transposes ----------------
    gb_pool = ctx.enter_context(tc.tile_pool(name=f"gb{it}", bufs=6))
    psu_pool = user_ctx.enter_context(tc.tile_pool(name=f"psu{it}", bufs=2, space="PSUM"))
    pst_pool = user_ctx.enter_context(tc.tile_pool(name=f"pst{it}", bufs=2, space="PSUM"))
    act_pool = ctx.enter_context(tc.tile_pool(name=f"actp{it}", bufs=2))
    actT = [const.tile([P, UCP], F32, name=f"actT{j}{it}") for j in range(2)]
    for m in range(KU):
        psu = psu_pool.tile([P, D], F32, name=f"psu{it}")
        nmm = R * KI * 2
        i = 0
        gbts = []
        for r in range(R):
            gb_t = gb_pool.tile([P, KI * P], BF16, name=f"gb_t{it}")
            nc.sync.dma_start(gb_t[:], t["gb"][m, r])
            gbts.append(gb_t)
        for r in range(R):
            for k in range(KI):
                for s in ("h", "l"):
                    nc.tensor.matmul(psu[:], gbts[r][:, k * P:(k + 1) * P],
                                     hi[s][(r, k)][:],
                                     start=(i == 0), stop=(i == nmm - 1))
                    i += 1
        z = z_pool.tile([P, D], F32, name=f"zu32{it}", tag="zu32")
        nc.vector.tensor_scalar_mul(z[:], psu[:], cu2[:, m:m + 1])
        act = act_pool.tile([P, D], F32, name=f"act{it}", tag="act")
        nc.vector.scalar_tensor_tensor(act[:], z[:], 0.1, z[:],
                                       _ALU.mult, _ALU.max)
        for j in range(2):
            psT = pst_pool.tile([P, P], F32, name=f"psT{it}")
            nc.tensor.transpose(psT[:], act[:, j * P:(j + 1) * P], ident[:])
            nc.vector.tensor_copy(actT[j][:, m * P:(m + 1) * P], psT[:])

    user_ctx.close()

    # ---------------- heads: sT = pT + yT (+fcb), row 64 = bu+gm ----------------
    head_ctx = _ES()
    pss_pool = head_ctx.enter_context(tc.tile_pool(name=f"pss{it}", bufs=2, space="PSUM"))
    sT = const.tile([O + 1, UCP], F32, name=f"sT{it}")
    for (c0, cw) in UCHUNKS:
        psS = pss_pool.tile([O, 512], F32, name=f"psS{it}", tag="pss")
        nmm = 2 + KH
        i = 0
        for kk in range(2):
            nc.tensor.matmul(psS[:, 0:cw], fcw[kk][:], actT[kk][:, c0:c0 + cw],
                             start=(i == 0), stop=(i == nmm - 1))
            i += 1
        for kh in range(KH):
            nc.tensor.matmul(psS[:, 0:cw], y0[kh][:], hist[kh][:, c0:c0 + cw],
                             start=(i == 0), stop=(i == nmm - 1))
            i += 1
        nc.scalar.activation(sT[0:O, c0:c0 + cw], psS[:, 0:cw],
                             mybir.ActivationFunctionType.Identity,
                             bias=fcb[:], scale=1.0)
    nc.vector.tensor_copy(sT[O:O + 1, :], burow[:])

    # ---------------- q head (after AllReduce) ----------------
    iag_pool = ctx.enter_context(tc.tile_pool(name=f"iag{it}", bufs=2))
    qT = const.tile([O + 1, IP], F32, name=f"qT{it}")
    qacts = []
    for kk in range(2):
        iag = iag_pool.tile([P, IP], F32, name=f"iag{it}", tag="iag")
        nc.sync.dma_start(iag[:], itemagg[kk * P:(kk + 1) * P, :])
        qact = iag_pool.tile([P, IP], F32, name=f"qact{kk}{it}", tag="qact")
        nc.vector.scalar_tensor_tensor(qact[:], iag[:], 0.1, iag[:],
                                       _ALU.mult, _ALU.max)
        qacts.append(qact)
    for (c0, cw) in ICHUNKS:
        psQ = pss_pool.tile([O, 512], F32, name=f"psQ{it}", tag="pss")
        for kk in range(2):
            nc.tensor.matmul(psQ[:, 0:cw], fcw[kk][:], qacts[kk][:, c0:c0 + cw],
                             start=(kk == 0), stop=(kk == 1))
        nc.scalar.activation(qT[0:O, c0:c0 + cw], psQ[:, 0:cw],
                             mybir.ActivationFunctionType.Identity,
                             bias=fcb[:], scale=1.0)
    nc.vector.tensor_copy(qT[O:O + 1, :], cirecip[:])

    head_ctx.close()

    # ---------------- final: out = D_ci (q'^T.T @ s'^T) + bi ----------------
    pso_pool = ctx.enter_context(tc.tile_pool(name=f"pso{it}", bufs=4, space="PSUM"))
    out_pool = ctx.enter_context(tc.tile_pool(name=f"outp{it}", bufs=2))
    for mi in range(KI):
        rows = min(P, I - mi * P)
        if rows <= 0:
            break
        for (c0, cw) in UCHUNKS:
            vw = min(cw, max(0, UC - c0))
            if vw <= 0:
                continue
            psO = pso_pool.tile([P, 512], F32, name=f"psO{it}")
            nc.tensor.matmul(psO[:, 0:cw], qT[:, mi * P:(mi + 1) * P],
                             sT[:, c0:c0 + cw], start=True, stop=True)
            out_t = out_pool.tile([P, 512], F32, name=f"out_t{it}")
            nc.scalar.activation(out_t[:, 0:cw], psO[:, 0:cw],
                                 mybir.ActivationFunctionType.Identity,
                                 bias=bi2[:, mi:mi + 1], scale=ci2[:, mi:mi + 1])
            nc.sync.dma_start(t["out"][mi * P:mi * P + rows, c0:c0 + vw],
                              out_t[0:rows, 0:vw])
    ctx.close()


_PROGRAM_CACHE = {}


def build_program(repeat=1):
    key = repeat
    if key in _PROGRAM_CACHE:
        return _PROGRAM_CACHE[key]
    nc = bacc.Bacc("TRN2", target_bir_lowering=False, debug=False,
                   num_devices=N_CORES)
    t = declare_io(nc)
    with tile.TileContext(nc) as tc:
        for it in range(repeat):
            emit_body(nc, tc, t, f"_i{it}" if repeat > 1 else "")
    nc.compile()
    _PROGRAM_CACHE[key] = (nc, t)
    return nc, t


def kernel(**inputs):
    in_maps = host_preprocess(**inputs)
    nc, _ = build_program()
    res = bass_utils.run_bass_kernel_spmd(
        nc, in_maps, core_ids=list(range(N_CORES)), trace=False)
    out = np.concatenate([res.results[c]["out"] for c in range(N_CORES)], axis=1)
    return out.astype(np.float32)


# revision 23
# speedup vs baseline: 514.8346x; 514.8346x over previous
"""Trainium2 Bass kernel for GCMC-style GNN message passing (nn_Net_6425271075083).

Strategy (8 NeuronCores, users sharded 1250/core):
  - Host converts the edge lists into dense per-rating adjacency count
    matrices (counts <= 3, exact in bf16) and the implicit-feedback
    index matrix into a per-user histogram; degrees -> cu/ci norm vectors.
  - Device does all the dense math:
      item side:  M_c^T = sum_r (D_cu (ufeat_c @ W_r))^T @ A_r[users_c]   [256,1024]
                  -> AllReduce over 8 cores -> item_agg^T
      user side:  user_agg_c = sum_r A_r[users_c]^T-layout @ (D_ci (ifeat @ W_r))
      heads:      p^T = fc_w^T @ leaky(user_agg * cu)^T (PE transposes)
                  y^T = Y0^T @ (Hist_c / sqrt_count)^T   (fused in same PSUM)
                  q^T = fc_w^T @ leaky(item_agg^T)  (ci deferred to final evict)
      final:      out_c = D_ci (q'^T.T @ s'^T) + bi, with the extra
                  q' row = 1/ci and s' row = bu + gm carrying the bias terms.
  - Big matmuls run as split-bf16 (hi+lo) against the exact-bf16 adjacency:
    validated rel-L2 vs fp32 reference ~6.5e-8.
"""
import os
import numpy as np
import ml_dtypes

import concourse.bass as bass
import concourse.bacc as bacc
import concourse.mybir as mybir
import concourse.tile as tile
from concourse import bass_utils
from concourse.masks import make_identity

BF = ml_dtypes.bfloat16
F32 = mybir.dt.float32
BF16 = mybir.dt.bfloat16

N_CORES = 8
U, I, R, D, O, H = 10000, 1000, 5, 256, 64, 1001
UC = U // N_CORES          # 1250
UCP = 1280                 # users per core, padded
IP = 1024                  # items padded
HP = 1024                  # hist bins padded
KU = UCP // 128            # 10 user k/m tiles
KI = IP // 128             # 8 item k/m tiles
KH = HP // 128             # 8 hist k tiles
UCHUNKS = [(0, 512), (512, 512), (1024, 256)]   # user free-dim chunks (padded)
ICHUNKS = [(0, 512), (512, 512)]                # item free-dim chunks

_ALU = mybir.AluOpType


def _split_bf16(x):
    hi = x.astype(BF)
    lo = (x - hi.astype(np.float32)).astype(BF)
    return hi, lo


def host_preprocess(src_idx, dst_idx, implicit_matrix, sqrt_count, global_mean,
                    ufeat, ifeat, W, fc_w, fc_b, bu, bi, Y):
    """Build per-core input maps (layout/sharding only plus degree/adjacency
    densification; all NN math happens on device)."""
    src = np.asarray(src_idx).astype(np.int64)
    dst = np.asarray(dst_idx).astype(np.int64)
    im = np.asarray(implicit_matrix).astype(np.int64)
    sqrt_count = np.asarray(sqrt_count, np.float32)
    gm = np.asarray(global_mean, np.float32).reshape(1)
    ufeat = np.asarray(ufeat, np.float32)
    ifeat = np.asarray(ifeat, np.float32)
    W = np.asarray(W, np.float32)
    fc_w = np.asarray(fc_w, np.float32)
    fc_b = np.asarray(fc_b, np.float32)
    bu = np.asarray(bu, np.float32)
    bi = np.asarray(bi, np.float32)
    Y = np.asarray(Y, np.float32)

    deg_u = np.bincount(src.reshape(-1), minlength=U).astype(np.float32)
    deg_i = np.bincount(dst.reshape(-1), minlength=I).astype(np.float32)
    cu = 1.0 / np.sqrt(np.maximum(deg_u, 1.0))
    ci = 1.0 / np.sqrt(np.maximum(deg_i, 1.0))
    def pack_cols(vec, ntiles):
        out = np.zeros((128, ntiles), np.float32)
        padded = np.zeros(128 * ntiles, np.float32)
        padded[:len(vec)] = vec
        out[:] = padded.reshape(ntiles, 128).T
        return out

    ci2 = pack_cols(ci, KI)
    bi2 = pack_cols(bi[:, 0], KI)
    cirecip_row = np.zeros((1, IP), np.float32)
    cirecip_row[0, :I] = 1.0 / ci

    # dense adjacency counts per rating [U, I]
    G = np.zeros((R, U, I), np.float32)
    for r in range(R):
        G[r] = np.bincount(src[r] * I + dst[r], minlength=U * I).reshape(U, I)

    # implicit histogram [U, H] with 1/sqrt_count folded
    hist = np.bincount((np.arange(U)[:, None] * H + im).reshape(-1),
                       minlength=U * H).reshape(U, H).astype(np.float32)
    histp = hist / sqrt_count

    Y0 = Y.copy()
    Y0[0] = 0.0
    y0_t = np.zeros((HP, O), np.float32)
    y0_t[:H] = Y0
    y0_t = y0_t.reshape(KH, 128, O)

    wh, wl = _split_bf16(W)                       # [5,256,256]
    ifT = np.zeros((D, IP), np.float32)
    ifT[:, :I] = ifeat.T
    ifh, ifl = _split_bf16(ifT)

    in_maps = []
    for c in range(N_CORES):
        us = slice(c * UC, (c + 1) * UC)
        # ga: [R, KU, 128, IP]  (lhs/rhs layout [users, items])
        ga = np.zeros((R, UCP, IP), BF)
        for r in range(R):
            ga[r, :UC, :I] = G[r][us].astype(BF)
        ga = ga.reshape(R, KU, 128, IP)
        # gb: [KU(m), R, 128(p=item-in-tile), KI*128(u)] from G^T
        gb = np.zeros((KU, R, 128, KI * 128), BF)
        for r in range(R):
            gt = np.zeros((IP, UCP), np.float32)
            gt[:I, :UC] = G[r][us].T
            # block for user-tile m: [IP, 128] -> [p, k*128+u]
            blocks = gt.reshape(KI, 128, KU, 128).transpose(2, 1, 0, 3)
            # blocks[m, p, k, u]
            gb[:, r] = blocks.reshape(KU, 128, KI * 128).astype(BF)

        ufT = np.zeros((D, UCP), np.float32)
        ufT[:, :UC] = ufeat[us].T
        ufh, ufl = _split_bf16(ufT)

        cu2 = pack_cols(cu[us], KU)
        bu_row = np.zeros((1, UCP), np.float32)
        bu_row[0, :UC] = bu[us, 0]

        hist_t = np.zeros((HP, UCP), np.float32)
        hist_t[:H, :UC] = histp[us].T
        hist_t = hist_t.reshape(KH, 128, UCP)

        in_maps.append({
            "ga": ga, "gb": gb,
            "uft_h": ufh, "uft_l": ufl,
            "ift_h": ifh, "ift_l": ifl,
            "w_h": wh, "w_l": wl,
            "fcw": fc_w.reshape(2, 128, O).copy(),
            "fcb": fc_b.reshape(O, 1).copy(),
            "y0t": y0_t, "histt": hist_t,
            "cu2": cu2, "ci2": ci2, "bi2": bi2,
            "cirecip": cirecip_row, "bu_row": bu_row,
            "gmv": gm.reshape(1, 1).copy(),
        })
    return in_maps


def declare_io(nc, timing_mode=False):
    t = {}
    def inp(name, shape, dt):
        t[name] = nc.dram_tensor(name, list(shape), dt, kind="ExternalInput").ap()
    inp("ga", (R, KU, 128, IP), BF16)
    inp("gb", (KU, R, 128, KI * 128), BF16)
    inp("uft_h", (D, UCP), BF16); inp("uft_l", (D, UCP), BF16)
    inp("ift_h", (D, IP), BF16); inp("ift_l", (D, IP), BF16)
    inp("w_h", (R, D, D), BF16); inp("w_l", (R, D, D), BF16)
    inp("fcw", (2, 128, O), F32)
    inp("fcb", (O, 1), F32)
    inp("y0t", (KH, 128, O), F32)
    inp("histt", (KH, 128, UCP), F32)
    inp("cu2", (128, KU), F32); inp("ci2", (128, KI), F32)
    inp("bi2", (128, KI), F32)
    inp("cirecip", (1, IP), F32); inp("bu_row", (1, UCP), F32)
    inp("gmv", (1, 1), F32)
    if timing_mode:
        t["tick"] = nc.dram_tensor("tick", [1, 4], F32, kind="ExternalOutput").ap()
    else:
        t["out"] = nc.dram_tensor("out", [I, UC], F32, kind="ExternalOutput").ap()
    return t


def emit_body(nc, tc, t, it, timing_mode=False, loop_mode=False):
    """Emit one full compute pass. `it` suffixes tile names for repeats."""
    from contextlib import ExitStack
    ctx = ExitStack()
    P = 128

    const = ctx.enter_context(tc.tile_pool(name=f"const{it}", bufs=1))

    def load_const(name, shape, dt, src_ap):
        tl = const.tile(shape, dt, name=f"{name}{it}")
        nc.sync.dma_start(tl[:], src_ap)
        return tl

    ident = const.tile([P, P], F32, name=f"ident{it}")
    make_identity(nc, ident[:])

    cu2 = load_const("cu2", [P, KU], F32, t["cu2"][:])
    ci2 = load_const("ci2", [P, KI], F32, t["ci2"][:])
    bi2 = load_const("bi2", [P, KI], F32, t["bi2"][:])
    cirecip = load_const("cirecip", [1, IP], F32, t["cirecip"][:])
    bu_in = load_const("bu_in", [1, UCP], F32, t["bu_row"][:])
    gmv = load_const("gmv", [1, 1], F32, t["gmv"][:])
    fcb = load_const("fcb", [O, 1], F32, t["fcb"][:])
    fcw = [load_const(f"fcw{k}", [P, O], F32, t["fcw"][k]) for k in range(2)]
    y0 = [load_const(f"y0_{k}", [P, O], F32, t["y0t"][k]) for k in range(KH)]
    hist = [load_const(f"hist{k}", [P, UCP], F32, t["histt"][k]) for k in range(KH)]
    uft = {s: [load_const(f"uft{s}{k}", [P, UCP], BF16,
                          t[f"uft_{s}"][k * P:(k + 1) * P, :]) for k in range(2)]
           for s in ("h", "l")}
    ift = {s: [load_const(f"ift{s}{k}", [P, IP], BF16,
                          t[f"ift_{s}"][k * P:(k + 1) * P, :]) for k in range(2)]
           for s in ("h", "l")}
    w = {s: [[load_const(f"w{s}{r}_{k}", [P, D], BF16,
                         t[f"w_{s}"][r, k * P:(k + 1) * P, :]) for k in range(2)]
             for r in range(R)]
         for s in ("h", "l")}

    burow = const.tile([1, UCP], F32, name=f"burow{it}")
    nc.vector.tensor_scalar_add(burow[:], bu_in[:], gmv[:, 0:1])

    # ---------------- item phase ----------------
    ga_pool = ctx.enter_context(tc.tile_pool(name=f"ga{it}", bufs=3))
    xw_pool = ctx.enter_context(tc.tile_pool(name=f"xw{it}", bufs=3))
    from contextlib import ExitStack as _ES
    item_ctx = _ES()
    psx_pool = item_ctx.enter_context(tc.tile_pool(name=f"psx{it}", bufs=2, space="PSUM"))
    psb_pool = item_ctx.enter_context(tc.tile_pool(name=f"psb{it}", bufs=1, space="PSUM"))

    psB = [[psb_pool.tile([P, 512], F32, name=f"psB{h}{cix}{it}")
            for cix in range(2)] for h in range(2)]
    n_rk = R * KU
    rk = 0
    for r in range(R):
        for k in range(KU):
            psx = psx_pool.tile([P, D], F32, name=f"psx{it}")
            mms = [(uft["h"][kk], w["h"][r][kk]) for kk in range(2)] + \
                  [(uft["l"][kk], w["h"][r][kk]) for kk in range(2)] + \
                  [(uft["h"][kk], w["l"][r][kk]) for kk in range(2)]
            for i, (lh, rh) in enumerate(mms):
                nc.tensor.matmul(psx[:], lh[:, k * P:(k + 1) * P], rh[:],
                                 start=(i == 0), stop=(i == len(mms) - 1))
            z32 = xw_pool.tile([P, D], F32, name=f"z32{it}", tag="z32")
            nc.vector.tensor_scalar_mul(z32[:], psx[:], cu2[:, k:k + 1])
            xh = xw_pool.tile([P, D], BF16, name=f"xh{it}", tag="xh")
            nc.vector.tensor_copy(xh[:], z32[:])
            xl = xw_pool.tile([P, D], BF16, name=f"xl{it}", tag="xl")
            nc.vector.scalar_tensor_tensor(xl[:], xh[:], -1.0, z32[:],
                                           _ALU.mult, _ALU.add)
            ga_t = ga_pool.tile([P, IP], BF16, name=f"ga_t{it}")
            nc.sync.dma_start(ga_t[:], t["ga"][r, k])
            for h in range(2):
                for cix, (c0, cw) in enumerate(ICHUNKS):
                    for x in (xh, xl):
                        nc.tensor.matmul(
                            psB[h][cix][:], x[:, h * P:(h + 1) * P],
                            ga_t[:, c0:c0 + cw],
                            start=(rk == 0 and x is xh),
                            stop=(rk == n_rk - 1 and x is xl))
            rk += 1

    mcT = [const.tile([P, IP], F32, name=f"mcT{h}{it}") for h in range(2)]
    for h in range(2):
        for cix, (c0, cw) in enumerate(ICHUNKS):
            nc.vector.tensor_copy(mcT[h][:, c0:c0 + cw], psB[h][cix][:])

    dram = ctx.enter_context(tc.tile_pool(name=f"dram{it}", bufs=1, space="DRAM"))
    itemp = dram.tile([D, IP], F32, name=f"itemp{it}")
    itemagg = dram.tile([D, IP], F32, name=f"itemagg{it}",
                        addr_space="Local" if loop_mode else "Shared")
    for h in range(2):
        nc.sync.dma_start(itemp[h * P:(h + 1) * P, :], mcT[h][:])
    if loop_mode:
        # collectives can't live inside control flow; equivalent-size DMA copy
        nc.gpsimd.dma_start(itemagg[:], itemp[:])
    else:
        nc.gpsimd.collective_compute(
            "AllReduce", _ALU.add,
            replica_groups=[list(range(N_CORES))],
            ins=[itemp.opt()], outs=[itemagg.opt()],
        )
    item_ctx.close()

    # ---------------- user phase: hi ----------------
    user_ctx = _ES()
    psh_pool = user_ctx.enter_context(tc.tile_pool(name=f"psh{it}", bufs=2, space="PSUM"))
    hi_pool = ctx.enter_context(tc.tile_pool(name=f"hi{it}", bufs=2 * R * KI))
    z_pool = ctx.enter_context(tc.tile_pool(name=f"zu{it}", bufs=3))
    hi = {"h": {}, "l": {}}
    for r in range(R):
        for k in range(KI):
            psh = psh_pool.tile([P, D], F32, name=f"psh{it}")
            mms = [(ift["h"][kk], w["h"][r][kk]) for kk in range(2)] + \
                  [(ift["l"][kk], w["h"][r][kk]) for kk in range(2)] + \
                  [(ift["h"][kk], w["l"][r][kk]) for kk in range(2)]
            for i, (lh, rh) in enumerate(mms):
                nc.tensor.matmul(psh[:], lh[:, k * P:(k + 1) * P], rh[:],
                                 start=(i == 0), stop=(i == len(mms) - 1))
            z32 = z_pool.tile([P, D], F32, name=f"zh{it}", tag="zh")
            nc.vector.tensor_scalar_mul(z32[:], psh[:], ci2[:, k:k + 1])
            hh = hi_pool.tile([P, D], BF16, name=f"hih{r}_{k}{it}", tag="hi")
            nc.vector.tensor_copy(hh[:], z32[:])
            hl = hi_pool.tile([P, D], BF16, name=f"hil{r}_{k}{it}", tag="hi")
            nc.vector.scalar_tensor_tensor(hl[:], hh[:], -1.0, z32[:],
                                           _ALU.mult, _ALU.add)
            hi["h"][(r, k)] = hh
            hi["l"][(r, k)] = hl

    # ---------------- user phase: user_agg + transposes ----------------
    gb_pool = ctx.enter_context(tc.tile_pool(name=f"gb{it}", bufs=6))
    psu_pool = user_ctx.enter_context(tc.tile_pool(name=f"psu{it}", bufs=2, space="PSUM"))
    pst_pool = user_ctx.enter_context(tc.tile_pool(name=f"pst{it}", bufs=2, space="PSUM"))
    act_pool = ctx.enter_context(tc.tile_pool(name=f"actp{it}", bufs=2))
    actT = [const.tile([P, UCP], F32, name=f"actT{j}{it}") for j in range(2)]
    for m in range(KU):
        psu = psu_pool.tile([P, D], F32, name=f"psu{it}")
        nmm = R * KI * 2
        i = 0
        gbts = []
        for r in range(R):
            gb_t = gb_pool.tile([P, KI * P], BF16, name=f"gb_t{it}")
            nc.sync.dma_start(gb_t[:], t["gb"][m, r])
            gbts.append(gb_t)
        for r in range(R):
            for k in range(KI):
                for s in ("h", "l"):
                    nc.tensor.matmul(psu[:], gbts[r][:, k * P:(k + 1) * P],
                                     hi[s][(r, k)][:],
                                     start=(i == 0), stop=(i == nmm - 1))
                    i += 1
        z = z_pool.tile([P, D], F32, name=f"zu32{it}", tag="zu32")
        nc.vector.tensor_scalar_mul(z[:], psu[:], cu2[:, m:m + 1])
        act = act_pool.tile([P, D], F32, name=f"act{it}", tag="act")
        nc.vector.scalar_tensor_tensor(act[:], z[:], 0.1, z[:],
                                       _ALU.mult, _ALU.max)
        for j in range(2):
            psT = pst_pool.tile([P, P], F32, name=f"psT{it}")
            nc.tensor.transpose(psT[:], act[:, j * P:(j + 1) * P], ident[:])
            nc.vector.tensor_copy(actT[j][:, m * P:(m + 1) * P], psT[:])

    user_ctx.close()

    # ---------------- heads: sT = pT + yT (+fcb), row 64 = bu+gm ----------------
    head_ctx = _ES()
    pss_pool = head_ctx.enter_context(tc.tile_pool(name=f"pss{it}", bufs=2, space="PSUM"))
    sT = const.tile([O + 1, UCP], F32, name=f"sT{it}")
    for (c0, cw) in UCHUNKS:
        psS = pss_pool.tile([O, 512], F32, name=f"psS{it}", tag="pss")
        nmm = 2 + KH
        i = 0
        for kk in range(2):
            nc.tensor.matmul(psS[:, 0:cw], fcw[kk][:], actT[kk][:, c0:c0 + cw],
                             start=(i == 0), stop=(i == nmm - 1))
            i += 1
        for kh in range(KH):
            nc.tensor.matmul(psS[:, 0:cw], y0[kh][:], hist[kh][:, c0:c0 + cw],
                             start=(i == 0), stop=(i == nmm - 1))
            i += 1
        nc.scalar.activation(sT[0:O, c0:c0 + cw], psS[:, 0:cw],
                             mybir.ActivationFunctionType.Identity,
                             bias=fcb[:], scale=1.0)
    nc.vector.tensor_copy(sT[O:O + 1, :], burow[:])

    # ---------------- q head (after AllReduce) ----------------
    iag_pool = ctx.enter_context(tc.tile_pool(name=f"iag{it}", bufs=2))
    qT = const.tile([O + 1, IP], F32, name=f"qT{it}")
    qacts = []
    for kk in range(2):
        iag = iag_pool.tile([P, IP], F32, name=f"iag{it}", tag="iag")
        nc.sync.dma_start(iag[:], itemagg[kk * P:(kk + 1) * P, :])
        qact = iag_pool.tile([P, IP], F32, name=f"qact{kk}{it}", tag="qact")
        nc.vector.scalar_tensor_tensor(qact[:], iag[:], 0.1, iag[:],
                                       _ALU.mult, _ALU.max)
        qacts.append(qact)
    for (c0, cw) in ICHUNKS:
        psQ = pss_pool.tile([O, 512], F32, name=f"psQ{it}", tag="pss")
        for kk in range(2):
            nc.tensor.matmul(psQ[:, 0:cw], fcw[kk][:], qacts[kk][:, c0:c0 + cw],
                             start=(kk == 0), stop=(kk == 1))
        nc.scalar.activation(qT[0:O, c0:c0 + cw], psQ[:, 0:cw],
                             mybir.ActivationFunctionType.Identity,
                             bias=fcb[:], scale=1.0)
    nc.vector.tensor_copy(qT[O:O + 1, :], cirecip[:])

    head_ctx.close()

    # ---------------- final: out = D_ci (q'^T.T @ s'^T) + bi ----------------
    if timing_mode:
        out_dst = dram.tile([I, UC], F32, name=f"outscratch{it}")
    else:
        out_dst = t["out"]
    pso_pool = ctx.enter_context(tc.tile_pool(name=f"pso{it}", bufs=4, space="PSUM"))
    out_pool = ctx.enter_context(tc.tile_pool(name=f"outp{it}", bufs=2))
    last_out_t = None
    for mi in range(KI):
        rows = min(P, I - mi * P)
        if rows <= 0:
            break
        for (c0, cw) in UCHUNKS:
            vw = min(cw, max(0, UC - c0))
            if vw <= 0:
                continue
            psO = pso_pool.tile([P, 512], F32, name=f"psO{it}")
            nc.tensor.matmul(psO[:, 0:cw], qT[:, mi * P:(mi + 1) * P],
                             sT[:, c0:c0 + cw], start=True, stop=True)
            out_t = out_pool.tile([P, 512], F32, name=f"out_t{it}")
            nc.scalar.activation(out_t[:, 0:cw], psO[:, 0:cw],
                                 mybir.ActivationFunctionType.Identity,
                                 bias=bi2[:, mi:mi + 1], scale=ci2[:, mi:mi + 1])
            nc.sync.dma_start(out_dst[mi * P:mi * P + rows, c0:c0 + vw],
                              out_t[0:rows, 0:vw])
            last_out_t = out_t
    if timing_mode:
        nc.sync.dma_start(t["tick"][:], last_out_t[0:1, 0:4])
    ctx.close()


_PROGRAM_CACHE = {}


def build_program(repeat=1, timing_mode=False):
    key = (repeat, timing_mode)
    if key in _PROGRAM_CACHE:
        return _PROGRAM_CACHE[key]
    nc = bacc.Bacc("TRN2", target_bir_lowering=False, debug=False,
                   num_devices=N_CORES)
    t = declare_io(nc, timing_mode)
    with tile.TileContext(nc) as tc:
        for it in range(repeat):
            emit_body(nc, tc, t, f"_i{it}" if repeat > 1 else "",
                      timing_mode=timing_mode)
    nc.compile()
    _PROGRAM_CACHE[key] = (nc, t)
    return nc, t


def build_loop_program(trips):
    key = ("loop", trips)
    if key in _PROGRAM_CACHE:
        return _PROGRAM_CACHE[key]
    nc = bacc.Bacc("TRN2", target_bir_lowering=False, debug=False,
                   num_devices=N_CORES)
    t = declare_io(nc, timing_mode=True)
    with tile.TileContext(nc) as tc:
        with tc.For_i(0, trips, 1):
            emit_body(nc, tc, t, "", timing_mode=True, loop_mode=True)
    nc.compile()
    _PROGRAM_CACHE[key] = (nc, t)
    return nc, t


def kernel(**inputs):
    in_maps = host_preprocess(**inputs)
    nc, _ = build_program()
    res = bass_utils.run_bass_kernel_spmd(
        nc, in_maps, core_ids=list(range(N_CORES)), trace=False)
    out = np.concatenate([res.results[c]["out"] for c in range(N_CORES)], axis=1)
    return out.astype(np.float32)


# revision 45
# speedup vs baseline: 991.1556x; 1.9252x over previous
"""Trainium2 Bass kernel for GCMC-style GNN message passing (nn_Net_6425271075083).

Strategy (8 NeuronCores, users sharded 1250/core):
  - Host converts the edge lists into dense per-rating adjacency count
    matrices (counts <= 3, exact in bf16) and the implicit-feedback
    index matrix into a per-user histogram; degrees -> cu/ci norm vectors.
  - Device does all the dense math:
      item side:  M_c^T = sum_r (D_cu (ufeat_c @ W_r))^T @ A_r[users_c]   [256,1024]
                  -> AllReduce over 8 cores -> item_agg^T
      user side:  user_agg_c = sum_r A_r[users_c]^T-layout @ (D_ci (ifeat @ W_r))
      heads:      p^T = fc_w^T @ leaky(user_agg * cu)^T (PE transposes)
                  y^T = Y0^T @ (Hist_c / sqrt_count)^T   (fused in same PSUM)
                  q^T = fc_w^T @ leaky(item_agg^T)  (ci deferred to final evict)
      final:      out_c = D_ci (q'^T.T @ s'^T) + bi, with the extra
                  q' row = 1/ci and s' row = bu + gm carrying the bias terms.
  - Big matmuls run as split-bf16 (hi+lo) against the exact-bf16 adjacency:
    validated rel-L2 vs fp32 reference ~6.5e-8.
"""
import os
import numpy as np
import ml_dtypes

import concourse.bass as bass
import concourse.bacc as bacc
import concourse.mybir as mybir
import concourse.tile as tile
from concourse import bass_utils
from concourse.masks import make_identity

BF = ml_dtypes.bfloat16
F32 = mybir.dt.float32
BF16 = mybir.dt.bfloat16

N_CORES = 8
U, I, R, D, O, H = 10000, 1000, 5, 256, 64, 1001
UC = U // N_CORES          # 1250
UCP = 1280                 # users per core, padded
IP = 1024                  # items padded
HP = 1024                  # hist bins padded
KU = UCP // 128            # 10 user k/m tiles
KI = IP // 128             # 8 item k/m tiles
KH = HP // 128             # 8 hist k tiles
UCHUNKS = [(0, 512), (512, 512), (1024, 256)]   # user free-dim chunks (padded)
ICHUNKS = [(0, 512), (512, 512)]                # item free-dim chunks

_ALU = mybir.AluOpType

# Precision config: split_g -> 2-term split-bf16 against exact adjacency;
# split_xw -> 3-term cross-split for the feature@W matmuls; hist_bf16 ->
# bf16 histogram/Y0 path (y is small vs p; error negligible).
CFG = {"split_g": True, "split_xw": True, "hist_bf16": False, "final_bf16": False}


def _cfg_key():
    return tuple(sorted(CFG.items()))


def _split_bf16(x):
    hi = x.astype(BF)
    lo = (x - hi.astype(np.float32)).astype(BF)
    return hi, lo


def host_preprocess(src_idx, dst_idx, implicit_matrix, sqrt_count, global_mean,
                    ufeat, ifeat, W, fc_w, fc_b, bu, bi, Y):
    """Build per-core input maps (layout/sharding only plus degree/adjacency
    densification; all NN math happens on device)."""
    src = np.asarray(src_idx).astype(np.int64)
    dst = np.asarray(dst_idx).astype(np.int64)
    im = np.asarray(implicit_matrix).astype(np.int64)
    sqrt_count = np.asarray(sqrt_count, np.float32)
    gm = np.asarray(global_mean, np.float32).reshape(1)
    ufeat = np.asarray(ufeat, np.float32)
    ifeat = np.asarray(ifeat, np.float32)
    W = np.asarray(W, np.float32)
    fc_w = np.asarray(fc_w, np.float32)
    fc_b = np.asarray(fc_b, np.float32)
    bu = np.asarray(bu, np.float32)
    bi = np.asarray(bi, np.float32)
    Y = np.asarray(Y, np.float32)

    deg_u = np.bincount(src.reshape(-1), minlength=U).astype(np.float32)
    deg_i = np.bincount(dst.reshape(-1), minlength=I).astype(np.float32)
    cu = 1.0 / np.sqrt(np.maximum(deg_u, 1.0))
    ci = 1.0 / np.sqrt(np.maximum(deg_i, 1.0))
    def pack_cols(vec, ntiles):
        out = np.zeros((128, ntiles), np.float32)
        padded = np.zeros(128 * ntiles, np.float32)
        padded[:len(vec)] = vec
        out[:] = padded.reshape(ntiles, 128).T
        return out

    ci2 = pack_cols(ci, KI)
    bi2 = pack_cols(bi[:, 0], KI)
    cirecip_row = np.zeros((1, IP), np.float32)
    cirecip_row[0, :I] = 1.0 / ci

    # dense adjacency counts per rating [U, I]
    G = np.zeros((R, U, I), np.float32)
    for r in range(R):
        G[r] = np.bincount(src[r] * I + dst[r], minlength=U * I).reshape(U, I)

    # implicit histogram [U, H] with 1/sqrt_count folded
    hist = np.bincount((np.arange(U)[:, None] * H + im).reshape(-1),
                       minlength=U * H).reshape(U, H).astype(np.float32)
    histp = hist / sqrt_count

    Y0 = Y.copy()
    Y0[0] = 0.0
    y0_t = np.zeros((HP, O), np.float32)
    y0_t[:H] = Y0
    y0_t = y0_t.reshape(KH, 128, O)
    hdtype = BF if CFG["hist_bf16"] else np.float32
    y0_t = y0_t.astype(hdtype)

    wh, wl = _split_bf16(W)                       # [5,256,256]
    ifT = np.zeros((D, IP), np.float32)
    ifT[:, :I] = ifeat.T
    ifh, ifl = _split_bf16(ifT)

    in_maps = []
    for c in range(N_CORES):
        us = slice(c * UC, (c + 1) * UC)
        # ga: [R, KU, 128, IP]  (lhs/rhs layout [users, items])
        ga = np.zeros((R, UCP, IP), BF)
        for r in range(R):
            ga[r, :UC, :I] = G[r][us].astype(BF)
        ga = ga.reshape(R, KU, 128, IP)
        # gb: [KU(m), R, 128(p=item-in-tile), KI*128(u)] from G^T
        gb = np.zeros((KU, R, 128, KI * 128), BF)
        for r in range(R):
            gt = np.zeros((IP, UCP), np.float32)
            gt[:I, :UC] = G[r][us].T
            # block for user-tile m: [IP, 128] -> [p, k*128+u]
            blocks = gt.reshape(KI, 128, KU, 128).transpose(2, 1, 0, 3)
            # blocks[m, p, k, u]
            gb[:, r] = blocks.reshape(KU, 128, KI * 128).astype(BF)

        ufT = np.zeros((D, UCP), np.float32)
        ufT[:, :UC] = ufeat[us].T
        ufh, ufl = _split_bf16(ufT)

        cu2 = pack_cols(cu[us], KU)
        bu_row = np.zeros((1, UCP), np.float32)
        bu_row[0, :UC] = bu[us, 0]

        hist_t = np.zeros((HP, UCP), np.float32)
        hist_t[:H, :UC] = histp[us].T
        hist_t = hist_t.reshape(KH, 128, UCP).astype(hdtype)

        in_maps.append({
            "ga": ga, "gb": gb,
            "uft_h": ufh, "uft_l": ufl,
            "ift_h": ifh, "ift_l": ifl,
            "w_h": wh, "w_l": wl,
            "fcw": fc_w.reshape(2, 128, O).copy(),
            "fcb": fc_b.reshape(O, 1).copy(),
            "y0t": y0_t, "histt": hist_t,
            "cu2": cu2, "ci2": ci2, "bi2": bi2,
            "cirecip": cirecip_row, "bu_row": bu_row,
            "gmv": gm.reshape(1, 1).copy(),
        })
    return in_maps


def declare_io(nc, timing_mode=False):
    t = {}
    def inp(name, shape, dt):
        t[name] = nc.dram_tensor(name, list(shape), dt, kind="ExternalInput").ap()
    inp("ga", (R, KU, 128, IP), BF16)
    inp("gb", (KU, R, 128, KI * 128), BF16)
    inp("uft_h", (D, UCP), BF16); inp("uft_l", (D, UCP), BF16)
    inp("ift_h", (D, IP), BF16); inp("ift_l", (D, IP), BF16)
    inp("w_h", (R, D, D), BF16); inp("w_l", (R, D, D), BF16)
    HDT = BF16 if CFG["hist_bf16"] else F32
    inp("fcw", (2, 128, O), F32)
    inp("fcb", (O, 1), F32)
    inp("y0t", (KH, 128, O), HDT)
    inp("histt", (KH, 128, UCP), HDT)
    inp("cu2", (128, KU), F32); inp("ci2", (128, KI), F32)
    inp("bi2", (128, KI), F32)
    inp("cirecip", (1, IP), F32); inp("bu_row", (1, UCP), F32)
    inp("gmv", (1, 1), F32)
    if timing_mode:
        t["tick"] = nc.dram_tensor("tick", [1, 4], F32, kind="ExternalOutput").ap()
    else:
        t["out"] = nc.dram_tensor("out", [I, UC], F32, kind="ExternalOutput").ap()
    return t


def emit_body(nc, tc, t, it, timing_mode=False, loop_mode=False):
    """Emit one full compute pass. `it` suffixes tile names for repeats."""
    from contextlib import ExitStack
    ctx = ExitStack()
    P = 128

    const = ctx.enter_context(tc.tile_pool(name=f"const{it}", bufs=1))

    def load_const(name, shape, dt, src_ap):
        tl = const.tile(shape, dt, name=f"{name}{it}")
        nc.gpsimd.dma_start(tl[:], src_ap)
        return tl

    ident = const.tile([P, P], F32, name=f"ident{it}")
    make_identity(nc, ident[:])

    cu2 = load_const("cu2", [P, KU], F32, t["cu2"][:])
    ci2 = load_const("ci2", [P, KI], F32, t["ci2"][:])
    bi2 = load_const("bi2", [P, KI], F32, t["bi2"][:])
    cirecip = load_const("cirecip", [1, IP], F32, t["cirecip"][:])
    bu_in = load_const("bu_in", [1, UCP], F32, t["bu_row"][:])
    gmv = load_const("gmv", [1, 1], F32, t["gmv"][:])
    fcb = load_const("fcb", [O, 1], F32, t["fcb"][:])
    fcw = [load_const(f"fcw{k}", [P, O], F32, t["fcw"][k]) for k in range(2)]
    HDT = BF16 if CFG["hist_bf16"] else F32
    y0 = [load_const(f"y0_{k}", [P, O], HDT, t["y0t"][k]) for k in range(KH)]
    hist = [load_const(f"hist{k}", [P, UCP], HDT, t["histt"][k]) for k in range(KH)]
    if CFG["hist_bf16"]:
        # keep every matmul in the pT/yT/qT PSUM groups uniformly bf16 —
        # mixing fp32 and bf16 matmuls in one accumulation group is unsafe
        fcw_b = []
        for k in range(2):
            fb = const.tile([P, O], BF16, name=f"fcwb{k}{it}")
            nc.vector.tensor_copy(fb[:], fcw[k][:])
            fcw_b.append(fb)
        head_fcw, ADT = fcw_b, BF16
    else:
        head_fcw, ADT = fcw, F32
    uft = {s: [load_const(f"uft{s}{k}", [P, UCP], BF16,
                          t[f"uft_{s}"][k * P:(k + 1) * P, :]) for k in range(2)]
           for s in ("h", "l")}
    ift = {s: [load_const(f"ift{s}{k}", [P, IP], BF16,
                          t[f"ift_{s}"][k * P:(k + 1) * P, :]) for k in range(2)]
           for s in ("h", "l")}
    w = {s: [[load_const(f"w{s}{r}_{k}", [P, D], BF16,
                         t[f"w_{s}"][r, k * P:(k + 1) * P, :]) for k in range(2)]
             for r in range(R)]
         for s in ("h", "l")}

    burow = const.tile([1, UCP], F32, name=f"burow{it}")
    nc.vector.tensor_scalar_add(burow[:], bu_in[:], gmv[:, 0:1])

    # ---------------- item phase ----------------
    ga_pool = ctx.enter_context(tc.tile_pool(name=f"ga{it}", bufs=3))
    xw_pool = ctx.enter_context(tc.tile_pool(name=f"xw{it}", bufs=3))
    from contextlib import ExitStack as _ES
    item_ctx = _ES()
    psx_pool = item_ctx.enter_context(tc.tile_pool(name=f"psx{it}", bufs=2, space="PSUM"))
    psb_pool = item_ctx.enter_context(tc.tile_pool(name=f"psb{it}", bufs=1, space="PSUM"))

    psB = [[psb_pool.tile([P, 512], F32, name=f"psB{h}{cix}{it}")
            for cix in range(2)] for h in range(2)]
    n_rk = R * KU
    rk = 0
    for r in range(R):
        for k in range(KU):
            psx = psx_pool.tile([P, D], F32, name=f"psx{it}")
            mms = [(uft["h"][kk], w["h"][r][kk]) for kk in range(2)]
            if CFG["split_xw"]:
                mms += [(uft["l"][kk], w["h"][r][kk]) for kk in range(2)] + \
                       [(uft["h"][kk], w["l"][r][kk]) for kk in range(2)]
            for i, (lh, rh) in enumerate(mms):
                nc.tensor.matmul(psx[:], lh[:, k * P:(k + 1) * P], rh[:],
                                 start=(i == 0), stop=(i == len(mms) - 1))
            if CFG["split_g"]:
                z32 = xw_pool.tile([P, D], F32, name=f"z32{it}", tag="z32")
                nc.vector.tensor_scalar_mul(z32[:], psx[:], cu2[:, k:k + 1])
                xh = xw_pool.tile([P, D], BF16, name=f"xh{it}", tag="xh")
                nc.vector.tensor_copy(xh[:], z32[:])
                xl = xw_pool.tile([P, D], BF16, name=f"xl{it}", tag="xl")
                nc.vector.scalar_tensor_tensor(xl[:], xh[:], -1.0, z32[:],
                                               _ALU.mult, _ALU.add)
                xs = (xh, xl)
            else:
                xh = xw_pool.tile([P, D], BF16, name=f"xh{it}", tag="xh")
                nc.vector.tensor_scalar_mul(xh[:], psx[:], cu2[:, k:k + 1])
                xs = (xh,)
            ga_t = ga_pool.tile([P, IP], BF16, name=f"ga_t{it}")
            nc.sync.dma_start(ga_t[:], t["ga"][r, k])
            for h in range(2):
                for cix, (c0, cw) in enumerate(ICHUNKS):
                    for x in xs:
                        nc.tensor.matmul(
                            psB[h][cix][:], x[:, h * P:(h + 1) * P],
                            ga_t[:, c0:c0 + cw],
                            start=(rk == 0 and x is xs[0]),
                            stop=(rk == n_rk - 1 and x is xs[-1]))
            rk += 1

    mcT = [const.tile([P, IP], F32, name=f"mcT{h}{it}") for h in range(2)]
    for h in range(2):
        for cix, (c0, cw) in enumerate(ICHUNKS):
            nc.vector.tensor_copy(mcT[h][:, c0:c0 + cw], psB[h][cix][:])

    dram = ctx.enter_context(tc.tile_pool(name=f"dram{it}", bufs=1, space="DRAM"))
    itemp = dram.tile([D, IP], F32, name=f"itemp{it}")
    itemagg = dram.tile([D, IP], F32, name=f"itemagg{it}",
                        addr_space="Local" if loop_mode else "Shared")
    for h in range(2):
        nc.sync.dma_start(itemp[h * P:(h + 1) * P, :], mcT[h][:])
    if loop_mode:
        # collectives can't live inside control flow; equivalent-size DMA copy
        nc.gpsimd.dma_start(itemagg[:], itemp[:])
    else:
        nc.gpsimd.collective_compute(
            "AllReduce", _ALU.add,
            replica_groups=[list(range(N_CORES))],
            ins=[itemp.opt()], outs=[itemagg.opt()],
        )
    item_ctx.close()

    # ---------------- user phase: hi ----------------
    user_ctx = _ES()
    psh_pool = user_ctx.enter_context(tc.tile_pool(name=f"psh{it}", bufs=2, space="PSUM"))
    n_hi = (2 if CFG["split_g"] else 1) * R * KI
    hi_pool = ctx.enter_context(tc.tile_pool(name=f"hi{it}", bufs=n_hi))
    z_pool = ctx.enter_context(tc.tile_pool(name=f"zu{it}", bufs=3))
    hi = {"h": {}, "l": {}}
    for r in range(R):
        for k in range(KI):
            psh = psh_pool.tile([P, D], F32, name=f"psh{it}")
            mms = [(ift["h"][kk], w["h"][r][kk]) for kk in range(2)]
            if CFG["split_xw"]:
                mms += [(ift["l"][kk], w["h"][r][kk]) for kk in range(2)] + \
                       [(ift["h"][kk], w["l"][r][kk]) for kk in range(2)]
            for i, (lh, rh) in enumerate(mms):
                nc.tensor.matmul(psh[:], lh[:, k * P:(k + 1) * P], rh[:],
                                 start=(i == 0), stop=(i == len(mms) - 1))
            if CFG["split_g"]:
                z32 = z_pool.tile([P, D], F32, name=f"zh{it}", tag="zh")
                nc.vector.tensor_scalar_mul(z32[:], psh[:], ci2[:, k:k + 1])
                hh = hi_pool.tile([P, D], BF16, name=f"hih{r}_{k}{it}", tag="hi")
                nc.vector.tensor_copy(hh[:], z32[:])
                hl = hi_pool.tile([P, D], BF16, name=f"hil{r}_{k}{it}", tag="hi")
                nc.vector.scalar_tensor_tensor(hl[:], hh[:], -1.0, z32[:],
                                               _ALU.mult, _ALU.add)
                hi["l"][(r, k)] = hl
            else:
                hh = hi_pool.tile([P, D], BF16, name=f"hih{r}_{k}{it}", tag="hi")
                nc.vector.tensor_scalar_mul(hh[:], psh[:], ci2[:, k:k + 1])
            hi["h"][(r, k)] = hh

    # ---------------- user phase: user_agg + transposes ----------------
    gb_pool = ctx.enter_context(tc.tile_pool(name=f"gb{it}", bufs=6))
    psu_pool = user_ctx.enter_context(tc.tile_pool(name=f"psu{it}", bufs=2, space="PSUM"))
    pst_pool = user_ctx.enter_context(tc.tile_pool(name=f"pst{it}", bufs=2, space="PSUM"))
    act_pool = ctx.enter_context(tc.tile_pool(name=f"actp{it}", bufs=2))
    actT = [const.tile([P, UCP], ADT, name=f"actT{j}{it}") for j in range(2)]
    for m in range(KU):
        psu = psu_pool.tile([P, D], F32, name=f"psu{it}")
        splits = ("h", "l") if CFG["split_g"] else ("h",)
        nmm = R * KI * len(splits)
        i = 0
        gbts = []
        for r in range(R):
            gb_t = gb_pool.tile([P, KI * P], BF16, name=f"gb_t{it}")
            nc.sync.dma_start(gb_t[:], t["gb"][m, r])
            gbts.append(gb_t)
        for r in range(R):
            for k in range(KI):
                for s in splits:
                    nc.tensor.matmul(psu[:], gbts[r][:, k * P:(k + 1) * P],
                                     hi[s][(r, k)][:],
                                     start=(i == 0), stop=(i == nmm - 1))
                    i += 1
        z = z_pool.tile([P, D], F32, name=f"zu32{it}", tag="zu32")
        nc.vector.tensor_scalar_mul(z[:], psu[:], cu2[:, m:m + 1])
        act = act_pool.tile([P, D], F32, name=f"act{it}", tag="act")
        nc.vector.scalar_tensor_tensor(act[:], z[:], 0.1, z[:],
                                       _ALU.mult, _ALU.max)
        for j in range(2):
            psT = pst_pool.tile([P, P], F32, name=f"psT{it}")
            nc.tensor.transpose(psT[:], act[:, j * P:(j + 1) * P], ident[:])
            nc.vector.tensor_copy(actT[j][:, m * P:(m + 1) * P], psT[:])

    user_ctx.close()

    # ---------------- heads: sT = pT + yT (+fcb), row 64 = bu+gm ----------------
    FDT = BF16 if CFG["final_bf16"] else F32
    head_ctx = _ES()
    pss_pool = head_ctx.enter_context(tc.tile_pool(name=f"pss{it}", bufs=2, space="PSUM"))
    sT = const.tile([O + 1, UCP], FDT, name=f"sT{it}")
    for (c0, cw) in UCHUNKS:
        psS = pss_pool.tile([O, 512], F32, name=f"psS{it}", tag="pss")
        nmm = 2 + KH
        i = 0
        for kk in range(2):
            nc.tensor.matmul(psS[:, 0:cw], head_fcw[kk][:], actT[kk][:, c0:c0 + cw],
                             start=(i == 0), stop=(i == nmm - 1))
            i += 1
        for kh in range(KH):
            nc.tensor.matmul(psS[:, 0:cw], y0[kh][:], hist[kh][:, c0:c0 + cw],
                             start=(i == 0), stop=(i == nmm - 1))
            i += 1
        nc.scalar.activation(sT[0:O, c0:c0 + cw], psS[:, 0:cw],
                             mybir.ActivationFunctionType.Identity,
                             bias=fcb[:], scale=1.0)
    nc.vector.tensor_copy(sT[O:O + 1, :], burow[:])

    # ---------------- q head (after AllReduce) ----------------
    iag_pool = ctx.enter_context(tc.tile_pool(name=f"iag{it}", bufs=2))
    qT = const.tile([O + 1, IP], FDT, name=f"qT{it}")
    qacts = []
    for kk in range(2):
        iag = iag_pool.tile([P, IP], F32, name=f"iag{it}", tag="iag")
        nc.sync.dma_start(iag[:], itemagg[kk * P:(kk + 1) * P, :])
        qact = iag_pool.tile([P, IP], ADT, name=f"qact{kk}{it}", tag="qact")
        nc.vector.scalar_tensor_tensor(qact[:], iag[:], 0.1, iag[:],
                                       _ALU.mult, _ALU.max)
        qacts.append(qact)
    for (c0, cw) in ICHUNKS:
        psQ = pss_pool.tile([O, 512], F32, name=f"psQ{it}", tag="pss")
        for kk in range(2):
            nc.tensor.matmul(psQ[:, 0:cw], head_fcw[kk][:], qacts[kk][:, c0:c0 + cw],
                             start=(kk == 0), stop=(kk == 1))
        nc.scalar.activation(qT[0:O, c0:c0 + cw], psQ[:, 0:cw],
                             mybir.ActivationFunctionType.Identity,
                             bias=fcb[:], scale=1.0)
    nc.vector.tensor_copy(qT[O:O + 1, :], cirecip[:])

    head_ctx.close()

    # ---------------- final: out = D_ci (q'^T.T @ s'^T) + bi ----------------
    if timing_mode:
        out_dst = dram.tile([I, UC], F32, name=f"outscratch{it}")
    else:
        out_dst = t["out"]
    pso_pool = ctx.enter_context(tc.tile_pool(name=f"pso{it}", bufs=4, space="PSUM"))
    out_pool = ctx.enter_context(tc.tile_pool(name=f"outp{it}", bufs=2))
    last_out_t = None
    for mi in range(KI):
        rows = min(P, I - mi * P)
        if rows <= 0:
            break
        for (c0, cw) in UCHUNKS:
            vw = min(cw, max(0, UC - c0))
            if vw <= 0:
                continue
            psO = pso_pool.tile([P, 512], F32, name=f"psO{it}")
            nc.tensor.matmul(psO[:, 0:cw], qT[:, mi * P:(mi + 1) * P],
                             sT[:, c0:c0 + cw], start=True, stop=True)
            out_t = out_pool.tile([P, 512], F32, name=f"out_t{it}")
            nc.scalar.activation(out_t[:, 0:cw], psO[:, 0:cw],
                                 mybir.ActivationFunctionType.Identity,
                                 bias=bi2[:, mi:mi + 1], scale=ci2[:, mi:mi + 1])
            nc.sync.dma_start(
                out_dst[mi * P:mi * P + rows, c0:c0 + vw], out_t[0:rows, 0:vw])
            last_out_t = out_t
    if timing_mode:
        nc.sync.dma_start(t["tick"][:], last_out_t[0:1, 0:4])
    ctx.close()


_PROGRAM_CACHE = {}


def build_program(repeat=1, timing_mode=False):
    key = (repeat, timing_mode, _cfg_key())
    if key in _PROGRAM_CACHE:
        return _PROGRAM_CACHE[key]
    nc = bacc.Bacc("TRN2", target_bir_lowering=False, debug=False,
                   num_devices=N_CORES)
    t = declare_io(nc, timing_mode)
    with tile.TileContext(nc) as tc:
        for it in range(repeat):
            emit_body(nc, tc, t, f"_i{it}" if repeat > 1 else "",
                      timing_mode=timing_mode)
    nc.compile()
    _PROGRAM_CACHE[key] = (nc, t)
    return nc, t


def build_loop_program(trips):
    key = ("loop", trips, _cfg_key())
    if key in _PROGRAM_CACHE:
        return _PROGRAM_CACHE[key]
    nc = bacc.Bacc("TRN2", target_bir_lowering=False, debug=False,
                   num_devices=N_CORES)
    t = declare_io(nc, timing_mode=True)
    with tile.TileContext(nc) as tc:
        with tc.For_i(0, trips, 1):
            emit_body(nc, tc, t, "", timing_mode=True, loop_mode=True)
    nc.compile()
    _PROGRAM_CACHE[key] = (nc, t)
    return nc, t


def kernel(**inputs):
    in_maps = host_preprocess(**inputs)
    nc, _ = build_program()
    res = bass_utils.run_bass_kernel_spmd(
        nc, in_maps, core_ids=list(range(N_CORES)), trace=False)
    out = np.concatenate([res.results[c]["out"] for c in range(N_CORES)], axis=1)
    return out.astype(np.float32)
